# revision 41
# baseline (speedup 1.0000x reference)
"""Trainium2 Bass kernel for nn_AttentionBlock (GroupNorm + 8-head self-attention + residual).

Full inputs in, full output out. Sharding: data-parallel over batch across the
8 NeuronCores (16 batches -> 2 per core), weights replicated, no collectives.

Layout strategy (per core, per batch; C=512 channels, S=1024 tokens):
  - x and xhat live channels-on-partitions; GroupNorm group stats are tiny PE
    matmuls against one-hot group matrices.  xhat is stored fp8 as channel-pair
    tiles [128, 2, S] feeding DoubleRow projections (2 fp8 rows per PE cell,
    K=256 per matmul).
  - All four projections (QKV + out) and P@V run as fp8e4m3 DoubleRow
    matmuls; operands are pre-scaled host-side (wq x16 incl 1/sqrt(dk),
    wk x16, wv x8, wo x8) to sit in e4m3's normal range, and the scales are
    folded back out through the free affine of exp (scale=1/256), the softmax
    denominator (x8 on the sums row), and the epilogue (x1/8).
  - scores are computed TRANSPOSED: scoresT[j, i] = k_j . q_i so the softmax
    reduction aligns with the matmul contraction; per-head K=64 contractions
    run as two concurrent row-group matmuls (rows 0-63 / 64-127 alternating).
  - exp() runs on ScalarE straight out of PSUM (the critical engine:
    S*S*heads elements at 1/lane/cycle); everything else is scheduled via a
    global work queue drained between score groups to hide under it.
  - P@V gives resU^T [65, S] per head (row 64 = softmax denominators);
    normalization is reciprocal_approx_fast + gpsimd partition_broadcast +
    one DVE mul writing the fp8 pair-tile the out-projection consumes.
  - residual-add is fused in the epilogue from the retained x tiles.
The softmax max-subtraction is skipped: scores stay in fp32 PSUM and an
exp-shift constant keeps exp() inside e4m3 range (max real score ~6.4).
"""

import numpy as np
import ml_dtypes

import concourse.bacc as bacc
import concourse.tile as tile
from concourse import mybir
from concourse.bass_utils import run_bass_kernel_spmd

N_CORES = 8
B, C, H, W = 16, 512, 32, 32
S = H * W                      # 1024
BL = B // N_CORES              # 2 batches per core
NH, DK = 8, 64
NG = 32                        # groupnorm groups
GSZ = C // NG                  # 16 channels per group
EPS = 1e-5
F32 = mybir.dt.float32
BF16 = mybir.dt.bfloat16
FP8 = mybir.dt.float8e4
DR = mybir.MatmulPerfMode.DoubleRow
AF = mybir.ActivationFunctionType
OP = mybir.AluOpType
NPBF16 = ml_dtypes.bfloat16
NPFP8 = ml_dtypes.float8_e4m3
# exp((s'/256) - EXP_SHIFT) keeps P inside e4m3 range (max ~240); the shift
# cancels exactly in the softmax normalization.  s' = 256*s from the x16
# pre-scales on wq and wk.
EXP_SHIFT = 1.5
QK_SCALE = 16.0
V_SCALE = 8.0
O_SCALE = 8.0

# test.py can flip these; results stashed in LAST.
TRACE = False
LAST = {}


def _build(has_bqk, has_bv, has_outb):
    nc = bacc.Bacc()

    x_d = nc.dram_tensor("x", [BL, C, S], F32, kind="ExternalInput")
    # DoubleRow channel-pair weight layout: [g, p, q, r] = w.T[g*256+q*128+p, r]
    wqt_d = nc.dram_tensor("wqt", [2, 128, 2, C], FP8, kind="ExternalInput")
    wkt_d = nc.dram_tensor("wkt", [2, 128, 2, C], FP8, kind="ExternalInput")
    wvt_d = nc.dram_tensor("wvt", [2, 128, 2, C], FP8, kind="ExternalInput")
    wot_d = nc.dram_tensor("wot", [2, 128, 2, C], FP8, kind="ExternalInput")
    g_d = nc.dram_tensor("gmat", [128, 8], F32, kind="ExternalInput")
    gt_d = nc.dram_tensor("gtmat", [8, 128], F32, kind="ExternalInput")
    bqk_d = (
        nc.dram_tensor("bqk", [128, 8], F32, kind="ExternalInput") if has_bqk else None
    )
    bv_d = nc.dram_tensor("bv", [1, C], FP8, kind="ExternalInput") if has_bv else None
    outb_d = (
        nc.dram_tensor("outb", [128, 4], F32, kind="ExternalInput") if has_outb else None
    )
    out_d = nc.dram_tensor("out", [BL, C, S], F32, kind="ExternalOutput")

    with tile.TileContext(nc) as tc:
        with (
            tc.tile_pool(name="const", bufs=1) as const,
            tc.tile_pool(name="px", bufs=8) as px,
            tc.tile_pool(name="pgn", bufs=4) as pgn,
            tc.tile_pool(name="pxh", bufs=5) as pxh,
            tc.tile_pool(name="pqt", bufs=8) as pqt,
            tc.tile_pool(name="pkt", bufs=8) as pkt,
            tc.tile_pool(name="pv", bufs=10) as pvp,
            tc.tile_pool(name="pexp", bufs=20) as pexp,
            tc.tile_pool(name="prec", bufs=4) as prec,
            tc.tile_pool(name="prt", bufs=5) as prt,
            tc.tile_pool(name="pout", bufs=3) as pout,
            tc.tile_pool(name="pps", bufs=3, space="PSUM") as pps,
            tc.tile_pool(name="psc", bufs=2, space="PSUM") as psc,
            tc.tile_pool(name="ppv", bufs=1, space="PSUM") as ppv,
        ):
            def load_x(b):
                # split each [128, S] tile into 4 partition-chunk DMAs so the
                # load spreads over 16 queues (a single monolithic DMA makes
                # the startup x-latency ~4x worse)
                xt = []
                for cb in range(4):
                    t = px.tile([128, S], F32, tag="x", name=f"x{b}_{cb}")
                    for pc in range(4):
                        nc.sync.dma_start(
                            out=t[pc * 32 : (pc + 1) * 32, :],
                            in_=x_d[
                                b, cb * 128 + pc * 32 : cb * 128 + (pc + 1) * 32, :
                            ],
                        )
                    xt.append(t)
                return xt

            xt0 = load_x(0)

            # ---- constants into SBUF (small groupnorm mats first)
            g_sb = const.tile([128, 8], F32, tag="g")
            nc.sync.dma_start(out=g_sb, in_=g_d[:, :])
            gt_sb = const.tile([8, 128], F32, tag="gt")
            nc.sync.dma_start(out=gt_sb, in_=gt_d[:, :])
            wq_sb, wk_sb, wv_sb, wo_sb = [], [], [], []
            for nm, lst, src in (
                ("q", wq_sb, wqt_d),
                ("k", wk_sb, wkt_d),
                ("v", wv_sb, wvt_d),
                ("o", wo_sb, wot_d),
            ):
                for g in range(2):
                    t = const.tile([128, 2, C], FP8, tag=f"w_{nm}_{g}")
                    nc.sync.dma_start(out=t, in_=src[g])
                    lst.append(t)
            shift_sb = const.tile([128, 1], F32, tag="shift")
            nc.vector.memset(shift_sb, -EXP_SHIFT)
            if has_bqk:
                bqk_sb = const.tile([128, 8], F32, tag="bqk")
                nc.sync.dma_start(out=bqk_sb, in_=bqk_d[:, :])
            if has_bv:
                bv_sb = const.tile([1, 2, C], FP8, tag="bv")
                nc.sync.dma_start(out=bv_sb[:, 0, :], in_=bv_d[:, :])
                ones_sb = const.tile([1, 2, S], FP8, tag="ones")
                nc.vector.memset(ones_sb, 1.0)
            if has_outb:
                outb_sb = const.tile([128, 4], F32, tag="outb")
                nc.sync.dma_start(out=outb_sb, in_=outb_d[:, :])

            # ---- PE warm-up with NO data dependencies (memset dummy): runs
            # at t~0 so the HAM clock gate opens before the first real matmul
            # and costs no startup latency.
            warm_sb = const.tile([128, 256], BF16, tag="warm")
            nc.vector.memset(warm_sb, 0.5)
            warm_ps = pps.tile([128, 256], F32, tag="pp", name="warm_ps")
            for wi in range(14):
                nc.tensor.matmul(
                    out=warm_ps,
                    lhsT=warm_sb[:, 0:128],
                    rhs=warm_sb,
                    start=True,
                    stop=True,
                )

            # ================= emission helpers =================
            def gn_batch(b, xt, xh):
                # groupnorm -> fp8 xhat channel-pair tiles [128, 2, S]; rstd
                # on DVE only (reciprocal seed + 2 Newton rsqrt steps) so
                # ScalarE never loads a non-Exp activation table.
                pgall = pps.tile([8, 4, 2], F32, tag="pp")   # [group, cb, (mean,e2)]
                for cb in range(4):
                    st6 = pgn.tile([128, 2, 6], F32, tag="st6")
                    nc.vector.bn_stats(out=st6[:, 0, :], in_=xt[cb][:, 0:512])
                    nc.vector.bn_stats(out=st6[:, 1, :], in_=xt[cb][:, 512:1024])
                    mv = pgn.tile([128, 2], F32, tag="mv")
                    nc.vector.bn_aggr(out=mv, in_=st6)
                    me2 = pgn.tile([128, 2], F32, tag="me2")
                    nc.vector.tensor_copy(out=me2[:, 0:1], in_=mv[:, 0:1])
                    nc.vector.tensor_tensor(
                        out=me2[:, 1:2], in0=mv[:, 0:1], in1=mv[:, 0:1], op=OP.mult
                    )
                    nc.vector.tensor_tensor(
                        out=me2[:, 1:2], in0=me2[:, 1:2], in1=mv[:, 1:2], op=OP.add
                    )
                    nc.tensor.matmul(
                        out=pgall[:, cb, :], lhsT=g_sb, rhs=me2, start=True, stop=True
                    )
                gm = pgn.tile([8, 4], F32, tag="gm")
                z = pgn.tile([8, 4], F32, tag="z")
                t2 = pgn.tile([8, 4], F32, tag="t2")
                y = pgn.tile([8, 4], F32, tag="y")
                nc.vector.tensor_scalar(
                    out=gm, in0=pgall[:, :, 0], scalar1=1.0 / GSZ, scalar2=None,
                    op0=OP.mult,
                )
                nc.vector.tensor_scalar(
                    out=z, in0=pgall[:, :, 1], scalar1=1.0 / GSZ, scalar2=EPS,
                    op0=OP.mult, op1=OP.add,
                )
                nc.vector.tensor_tensor(out=t2, in0=gm, in1=gm, op=OP.mult)
                nc.vector.tensor_tensor(out=z, in0=z, in1=t2, op=OP.subtract)
                # rsqrt(z): y0 = 1/z, then y <- y*(1.5 - 0.5*z*y^2) twice
                nc.vector.reciprocal(out=y, in_=z)
                for _ in range(2):
                    nc.vector.tensor_tensor(out=t2, in0=z, in1=y, op=OP.mult)
                    nc.vector.tensor_tensor(out=t2, in0=t2, in1=y, op=OP.mult)
                    nc.vector.tensor_scalar(
                        out=t2, in0=t2, scalar1=-0.5, scalar2=1.5,
                        op0=OP.mult, op1=OP.add,
                    )
                    nc.vector.tensor_tensor(out=y, in0=y, in1=t2, op=OP.mult)
                gs2 = pgn.tile([8, 2, 4], F32, tag="gs2")   # [(mean,rstd), cb]
                nc.vector.tensor_copy(out=gs2[:, 0, :], in_=gm)
                nc.vector.tensor_copy(out=gs2[:, 1, :], in_=y)
                for g in range(2):
                    t = pxh.tile([128, 2, S], FP8, tag="xh", name=f"xh{b}_{g}")
                    xh.append(t)
                for cb in range(4):
                    pb = pps.tile([128, 2], F32, tag="pp")
                    nc.tensor.matmul(
                        out=pb, lhsT=gt_sb, rhs=gs2[:, :, cb], start=True, stop=True
                    )
                    nc.vector.tensor_scalar(
                        out=xh[cb // 2][:, cb % 2, :],
                        in0=xt[cb],
                        scalar1=pb[:, 0:1],
                        scalar2=pb[:, 1:2],
                        op0=OP.subtract,
                        op1=OP.mult,
                    )

            def v_group(b, xh, vt, st):
                # V rows (x V_SCALE) for one 128-token S-tile, into the
                # jb-pair fp8 tile [128, 2, NH, 72] (pair dim q = st%2 feeds
                # the DoubleRow P@V contraction; 72-col head stride keeps the
                # pair-dim byte step 16-aligned; col 64 is the ones column).
                pv = pps.tile([128, 512], F32, tag="pp")
                for g in range(2):
                    nc.tensor.matmul(
                        out=pv,
                        lhsT=xh[g][:, :, st * 128 : (st + 1) * 128],
                        rhs=wv_sb[g],
                        start=(g == 0),
                        stop=(g == 1 and not has_bv),
                        perf_mode=DR,
                    )
                if has_bv:
                    nc.tensor.matmul(
                        out=pv,
                        lhsT=ones_sb[:, :, st * 128 : (st + 1) * 128],
                        rhs=bv_sb,
                        start=False,
                        stop=True,
                        perf_mode=DR,
                    )
                if st % 2 == 0:
                    t = pvp.tile([128, 2, NH, 72], FP8, tag="v", name=f"v{b}_{st}")
                    vt.append(t)
                t = vt[st // 2]
                q = st % 2
                nc.vector.memset(t[:, q, :, 64:65], 1.0)
                nc.vector.tensor_copy(
                    out=t[:, q, :, 0:64], in_=pv.rearrange("p (h d) -> p h d", h=NH)
                )

            def qk_units(b, xh, dst, w_sb, boff, rb):
                # one projection psum row-block -> one [128, S] bf16 head-pair
                # tile (scores stay bf16: fp8 gives no score-matmul speedup
                # without DoubleRow, and DoubleRow K=32 row-tiles measured
                # slower).  Two 512-column half-units so fill slots stay small.
                holder = {}
                pool = pqt if boff == 0 else pkt
                pfx = "q" if boff == 0 else "k"

                def half(sc):
                    if "t" not in holder:
                        t = pool.tile([128, S], BF16, tag="qk", name=f"{pfx}{b}_{rb}")
                        holder["t"] = t
                        dst.append(t)
                    t = holder["t"]
                    pq = pps.tile([128, 512], F32, tag="pp")
                    for g in range(2):
                        nc.tensor.matmul(
                            out=pq,
                            lhsT=w_sb[g][:, :, rb * 128 : (rb + 1) * 128],
                            rhs=xh[g][:, :, sc * 512 : (sc + 1) * 512],
                            start=(g == 0),
                            stop=(g == 1),
                            perf_mode=DR,
                        )
                    cols = slice(sc * 512, (sc + 1) * 512)
                    if has_bqk:
                        nc.vector.tensor_scalar_add(
                            out=t[:, cols],
                            in0=pq,
                            scalar1=bqk_sb[:, boff + rb : boff + rb + 1],
                        )
                    else:
                        nc.vector.tensor_copy(out=t[:, cols], in_=pq)

                return [lambda: half(0), lambda: half(1)]

            def epi_units(b, xt, rt2, cb):
                # out-projection (DoubleRow over fp8 result pairs) + residual;
                # two 512-column half-units (output DMA on the 2nd).
                holder = {}

                def half(sc):
                    if "t" not in holder:
                        holder["t"] = pout.tile(
                            [128, S], F32, tag="ot", name=f"ot{b}_{cb}"
                        )
                    ot = holder["t"]
                    po = pps.tile([128, 512], F32, tag="pp")
                    for g in range(2):
                        nc.tensor.matmul(
                            out=po,
                            lhsT=wo_sb[g][:, :, cb * 128 : (cb + 1) * 128],
                            rhs=rt2[g][:, :, sc * 512 : (sc + 1) * 512],
                            start=(g == 0),
                            stop=(g == 1),
                            perf_mode=DR,
                        )
                    cols = slice(sc * 512, (sc + 1) * 512)
                    nc.vector.scalar_tensor_tensor(
                        out=ot[:, cols],
                        in0=po,
                        scalar=1.0 / O_SCALE,
                        in1=xt[cb][:, cols],
                        op0=OP.mult,
                        op1=OP.add,
                    )
                    if has_outb:
                        nc.vector.tensor_scalar_add(
                            out=ot[:, cols],
                            in0=ot[:, cols],
                            scalar1=outb_sb[:, cb : cb + 1],
                        )
                    if sc == 1:
                        # chunked store: 4 queues per tile
                        for pc in range(4):
                            nc.sync.dma_start(
                                out=out_d[
                                    b,
                                    cb * 128 + pc * 32 : cb * 128 + (pc + 1) * 32,
                                    :,
                                ],
                                in_=ot[pc * 32 : (pc + 1) * 32, :],
                            )

                return [lambda: half(0), lambda: half(1)]

            # work queue of (pe_cost_ns, fn): fill() drains by estimated PE
            # cost so each inter-jb burst stays small and ScalarE (exec queue
            # depth 0) never starves behind a long PE burst.
            queue = []

            def fill(budget=1400.0):
                spent = 0.0
                while queue and spent < budget:
                    cost, fn = queue.pop(0)
                    fn()
                    spent += cost

            def attn_phaseA(b, qt, kt, hp, inline_pb=None):
                # transposed scores for both heads of the pair via concurrent
                # DoubleRow row-group matmuls (Ki=32 x Ko=2 each, heads at
                # partition bases 64*(hp%2)+{0,32} of the head-quad tile),
                # exp per head/jb into fp8 pair tiles.  P@V/normalize are
                # queued separately (pb_units) so they interleave with the
                # NEXT pair's scores and ScalarE never starves.
                ex = [[None] * 4, [None] * 4]
                pvh = [None, None]
                if inline_pb is not None:
                    # last pair: 4 accumulators live through phase A; by now
                    # the projection pools are idle, so borrow pps for 3
                    vt_i, rt2_i = inline_pb
                    pvh[0] = [
                        ppv.tile([65, 512], F32, tag="ppvt", name="ipv00"),
                        pps.tile([65, 512], F32, tag="pp", name="ipv01"),
                    ]
                    pvh[1] = [
                        pps.tile([65, 512], F32, tag="pp", name="ipv10"),
                        pps.tile([65, 512], F32, tag="pp", name="ipv11"),
                    ]
                for jb in range(8):
                    jp, q = jb // 2, jb % 2
                    if q == 0:
                        for hi in range(2):
                            e = pexp.tile(
                                [128, 2, S], FP8, tag="ex", name=f"ex{hi}"
                            )
                            ex[hi][jp] = e
                    # emit the 4 score matmuls alternating row groups
                    # (head-even rows 0-63, head-odd rows 64-127) so each
                    # LDWEIGHTS pulls ahead of the other group's in-flight
                    # matmul and the pair computes concurrently on the PE.
                    pss = [
                        psc.tile([128, S], F32, tag="ps", name=f"ps{hi}")
                        for hi in range(2)
                    ]
                    def smm(hi, sc):
                        prng = slice(hi * 64, (hi + 1) * 64)
                        cols = slice(sc * 512, (sc + 1) * 512)
                        nc.tensor.matmul(
                            out=pss[hi][:, cols],
                            lhsT=kt[hp][prng, jb * 128 : (jb + 1) * 128],
                            rhs=qt[hp][prng, cols],
                            start=True,
                            stop=True,
                        )

                    def sexp(hi):
                        nc.scalar.activation(
                            out=ex[hi][jp][:, q, :], in_=pss[hi], func=AF.Exp,
                            bias=shift_sb[:, 0:1], scale=1.0 / (QK_SCALE * QK_SCALE),
                        )

                    # head-even finishes after 3 matmuls so its exp starts
                    # while head-odd's last matmul still streams
                    smm(0, 0)
                    smm(1, 0)
                    smm(0, 1)
                    sexp(0)
                    smm(1, 1)
                    sexp(1)
                    if inline_pb is not None and q == 1:
                        # last pair: fold P@V into phase A so the kernel tail
                        # is only normalize+epilogue
                        for hi in range(2):
                            for sc in range(2):
                                nc.tensor.matmul(
                                    out=pvh[hi][sc],
                                    lhsT=vt_i[jp][:, :, 2 * hp + hi, 0:65],
                                    rhs=ex[hi][jp][:, :, sc * 512 : (sc + 1) * 512],
                                    start=(jp == 0),
                                    stop=(jp == 3),
                                    perf_mode=DR,
                                )
                    fill()
                if inline_pb is not None:
                    for hi in range(2):
                        for sc in range(2):
                            norm_sc(rt2_i, 2 * hp + hi, sc, pvh[hi][sc])
                return ex

            def norm_sc(rt2, h, sc, pvt):
                g, qq, prow = h // 4, (h // 2) % 2, (h % 2) * 64
                # scale the sums row by V_SCALE so the x8 on V cancels;
                # stage to SBUF (the custom-DVE recip reads garbage from
                # PSUM on hardware)
                stage = prec.tile([1, 512], F32, tag="st")
                nc.vector.tensor_scalar(
                    out=stage, in0=pvt[64:65, :], scalar1=V_SCALE,
                    scalar2=None, op0=OP.mult,
                )
                rrow = prec.tile([1, 512], F32, tag="rr")
                nc.vector.reciprocal_approx_fast(out=rrow, in_=stage)
                rbt = prec.tile([64, 512], F32, tag="rb")
                nc.gpsimd.partition_broadcast(rbt, rrow)
                nc.vector.tensor_tensor(
                    out=rt2[g][prow : prow + 64, qq, sc * 512 : (sc + 1) * 512],
                    in0=pvt[0:64, :],
                    in1=rbt,
                    op=OP.mult,
                )

            def pb_units(b, vt, rt2, hp, ex):
                # P@V + normalize, one sc column-half at a time (single ppv
                # accumulator buf), as 12 small fill units per pair
                units = []
                for hi in range(2):
                    h = 2 * hp + hi
                    for sc in range(2):
                        holder = {}

                        def pv_jps(lo, hi_=None, sc_=None, hh=None, hld=None):
                            # DoubleRow P@V: each matmul contracts a 256-token
                            # jb-pair (2 fp8 rows per PE cell)
                            if "p" not in hld:
                                hld["p"] = ppv.tile(
                                    [65, 512], F32, tag="ppvt", name="pvt"
                                )
                            for jp in range(lo, lo + 2):
                                nc.tensor.matmul(
                                    out=hld["p"],
                                    lhsT=vt[jp][:, :, hh, 0:65],
                                    rhs=ex[hi_][jp][
                                        :, :, sc_ * 512 : (sc_ + 1) * 512
                                    ],
                                    start=(jp == 0),
                                    stop=(jp == 3),
                                    perf_mode=DR,
                                )

                        units.append(
                            (900, lambda hi_=hi, sc_=sc, hh=h, hld=holder: pv_jps(
                                0, hi_=hi_, sc_=sc_, hh=hh, hld=hld
                            ))
                        )
                        units.append(
                            (900, lambda hi_=hi, sc_=sc, hh=h, hld=holder: pv_jps(
                                2, hi_=hi_, sc_=sc_, hh=hh, hld=hld
                            ))
                        )
                        units.append(
                            (150, lambda hi_=hi, sc_=sc, hh=h, hld=holder: norm_sc(
                                rt2, hh, sc_, hld["p"]
                            ))
                        )
                return units

            # ================= schedule =================
            # batch-0 prep emitted directly; everything else (batch-1 x DMA,
            # V tiles, remaining projections, batch-1 groupnorm, P@V+normalize
            # of the previous pair, epilogues) drains from one global work
            # queue two units per jb inside the attention loops, so ScalarE
            # streams exps continuously.  P@V units for pair p are PREPENDED
            # when pair p+1 starts so they run early.
            xh0, qt0, kt0, vt0 = [], [], [], []
            gn_batch(0, xt0, xh0)
            # pair-0/1 row-blocks of Q/K emitted directly: each pair's Q/K
            # must be materialized by the time its phase A is emitted.
            for rb in range(2):
                for u in qk_units(0, xh0, qt0, wq_sb, 0, rb):
                    u()
                for u in qk_units(0, xh0, kt0, wk_sb, 4, rb):
                    u()

            xt1 = []
            xh1, qt1, kt1, vt1 = [], [], [], []
            queue.append((1, lambda: xt1.extend(load_x(1))))
            for st in range(8):
                queue.append((900, lambda st=st: v_group(0, xh0, vt0, st)))
            for rb in range(2, 4):
                queue.extend((860, u) for u in qk_units(0, xh0, qt0, wq_sb, 0, rb))
                queue.extend((860, u) for u in qk_units(0, xh0, kt0, wk_sb, 4, rb))
            # batch-1 groupnorm deferred into the queue: its PSUM tiles must
            # sit BEHIND batch-0's V/QK in the pps ring, else the in-order PE
            # head-of-line blocks on xt1's DMA through the pool WAR edge.
            queue.append((1000, lambda: gn_batch(1, xt1, xh1)))
            for rb in range(2):
                queue.extend((860, u) for u in qk_units(1, xh1, qt1, wq_sb, 0, rb))
                queue.extend((860, u) for u in qk_units(1, xh1, kt1, wk_sb, 4, rb))
            for st in range(8):
                queue.append((900, lambda st=st: v_group(1, xh1, vt1, st)))
            for rb in range(2, 4):
                queue.extend((860, u) for u in qk_units(1, xh1, qt1, wq_sb, 0, rb))
                queue.extend((860, u) for u in qk_units(1, xh1, kt1, wk_sb, 4, rb))

            rt0 = [
                prt.tile([128, 2, S], FP8, tag="rt", name=f"rt0_{g}") for g in range(2)
            ]
            rt1 = [
                prt.tile([128, 2, S], FP8, tag="rt", name=f"rt1_{g}") for g in range(2)
            ]
            for hp in range(4):
                ex = attn_phaseA(0, qt0, kt0, hp)
                queue[:0] = pb_units(0, vt0, rt0, hp, ex)
            for cb in range(4):
                queue.extend((900, u) for u in epi_units(0, xt0, rt0, cb))
            for hp in range(4):
                if hp == 3:
                    # drain any remaining queued work (pps users) before the
                    # inline pair borrows the projection PSUM bufs
                    fill(1e9)
                ex = attn_phaseA(
                    1, qt1, kt1, hp,
                    inline_pb=(vt1, rt1) if hp == 3 else None,
                )
                if hp < 3:
                    queue[:0] = pb_units(1, vt1, rt1, hp, ex)
            fill(1e9)
            for cb in range(4):
                for u in epi_units(1, xt1, rt1, cb):
                    u()

    nc.finalize()
    return nc


def _prep(inputs):
    """Host-side weight prep shared by kernel() and simtest."""
    norm_w = np.asarray(inputs["norm_w"], np.float64)
    norm_b = np.asarray(inputs["norm_b"], np.float64)
    proj_w = np.asarray(inputs["proj_w"], np.float64)
    proj_b = np.asarray(inputs["proj_b"], np.float64)
    out_w = np.asarray(inputs["out_w"], np.float64)
    out_b = np.asarray(inputs["out_b"], np.float32)

    # split qkv rows (row = h*192 + t*64 + d, t in {q,k,v}) into head-major mats
    pw = proj_w.reshape(NH, 3, DK, C)
    pb = proj_b.reshape(NH, 3, DK)
    mats, biases = [], []
    for t in range(3):
        wm = pw[:, t].reshape(NH * DK, C)
        bv = pb[:, t].reshape(NH * DK)
        # fold groupnorm affine: y = xhat*nw + nb  =>  W@y + b = (W*nw)@xhat + (W@nb + b)
        mats.append(wm * norm_w[None, :])
        biases.append(bv + wm @ norm_b)
    wq, wk, wv = mats
    bq, bk, bv = biases
    scale = DK ** -0.5
    wq = wq * scale * QK_SCALE
    bq = bq * scale * QK_SCALE
    wk = wk * QK_SCALE
    bk = bk * QK_SCALE

    wv = wv * V_SCALE
    bv = bv * V_SCALE
    wo = np.asarray(out_w, np.float64) * O_SCALE

    def pack(wmat):
        # [rows, C] -> transpose -> [c_in, rows] -> [g, p, q, rows]
        wT = np.ascontiguousarray(wmat.T)  # [C, rows]
        return np.ascontiguousarray(
            wT.reshape(2, 2, 128, wT.shape[1]).transpose(0, 2, 1, 3)
        ).astype(NPFP8)

    G = np.zeros((128, 8), np.float32)
    G[np.arange(128), np.arange(128) // GSZ] = 1.0

    has_bqk = bool(np.any(bq) or np.any(bk))
    has_bv = bool(np.any(bv))
    has_outb = bool(np.any(out_b))

    m = {
        "wqt": pack(wq),
        "wkt": pack(wk),
        "wvt": pack(wv),
        "wot": pack(wo),
        "gmat": G,
        "gtmat": np.ascontiguousarray(G.T),
    }
    if has_bqk:
        bqk = np.zeros((128, 8), np.float32)
        bqk[:, 0:4] = bq.reshape(4, 128).T
        bqk[:, 4:8] = bk.reshape(4, 128).T
        m["bqk"] = bqk
    if has_bv:
        m["bv"] = np.ascontiguousarray(bv.reshape(1, C)).astype(NPFP8)
    if has_outb:
        m["outb"] = np.ascontiguousarray(out_b.reshape(4, 128).T)
    return m, (has_bqk, has_bv, has_outb)


def kernel(**inputs):
    x = np.asarray(inputs["x"], np.float32)
    common, flags = _prep(inputs)
    nc = _build(*flags)

    xr = x.reshape(B, C, S)
    in_maps = []
    for c in range(N_CORES):
        m = dict(common)
        m["x"] = np.ascontiguousarray(xr[c * BL : (c + 1) * BL])
        in_maps.append(m)

    # guard: bass_utils imports antenv.axon_hooks when tracing is requested
    # (e.g. via BASS_TRACE env); provide a no-op module if the image lacks it.
    try:
        import antenv.axon_hooks  # noqa: F401
    except ImportError:
        import sys
        import types

        import antenv

        _m = types.ModuleType("antenv.axon_hooks")
        _m._hook = None
        _m.set_axon_ntff_profile_hook = lambda h: setattr(_m, "_hook", h)
        _m.get_axon_ntff_profile_hook = lambda: _m._hook
        sys.modules["antenv.axon_hooks"] = _m
        antenv.axon_hooks = _m

    res = None
    for attempt in range(3):
        try:
            res = run_bass_kernel_spmd(
                nc, in_maps, core_ids=list(range(N_CORES)), trace=TRACE
            )
            break
        except Exception:
            # transient NRT_EXEC_UNIT_UNRECOVERABLE-style device hiccups
            # clear on retry; re-raise on the final attempt
            if attempt == 2:
                raise
    LAST["exec_time_ns"] = res.exec_time_ns
    LAST["mean_exec_time_ns"] = res.mean_exec_time_ns
    LAST["result"] = res

    out = np.concatenate([res.results[c]["out"] for c in range(N_CORES)], axis=0)
    return np.ascontiguousarray(out.reshape(B, C, H, W).astype(np.float32))
